# revision 9
# baseline (speedup 1.0000x reference)
"""Int-infer matmul kernel for trn2, 8 NeuronCores, data-parallel over (b,h).

reference: y = clip(round(matmul(clip(round(x1*r1)), clip(round(x2*r2))) / 16), -128, 127)
shapes: x1 [2,16,2048,64] f32, x2 [2,16,64,2048] f32 -> y [2,16,2048,2048] f32

Strategy (per core, 4 of the 32 (b,h) pairs):
 - rescale: f32 -> *r -> int8 (HW convert = RNE + saturate == clip(round(.)))
 - int8 -> bf16 (exact for [-128,127]); bf16 matmul accumulates exactly in f32 PSUM
 - x1 transposed on PE (col-tiled so pair A -> psum partitions 0:64, B -> 64:128)
 - main matmuls row-packed: two K=64 matmuls (pairs A,B) run concurrently via
   tile_position (0,0)/(64,0)
 - evict psum f32 -> *1/16 -> int8 (RNE+sat == clip(round(y/16))), alternating
   DVE/ACT; int8 output DMA'd out (4x fewer bytes), upcast to f32 on host
"""
import sys

sys.path.insert(0, "/opt/trn_rl_repo")

import numpy as np
import concourse.bass as bass
import concourse.bacc as bacc
import concourse.mybir as mybir
import concourse.tile as tile
from concourse.bass_utils import run_bass_kernel_spmd
from concourse.masks import make_identity

F32 = mybir.dt.float32
BF16 = mybir.dt.bfloat16
I8 = mybir.dt.int8
AF = mybir.ActivationFunctionType

N_CORES = 8
PAIRS_PER_CORE = 4  # 2*16 = 32 (b,h) pairs / 8 cores
S = 2048
D = 64
N_MM = 512  # moving free dim per matmul
INV_G = 1.0 / 16.0


def build_program(r1: float, r2: float, repeat: int = 1) -> bass.Bass:
    nc = bacc.Bacc("TRN2", target_bir_lowering=False, debug=False, num_devices=N_CORES)
    x1 = nc.dram_tensor("x1", [PAIRS_PER_CORE, S, D], F32, kind="ExternalInput").ap()
    x2 = nc.dram_tensor("x2", [PAIRS_PER_CORE, D, S], F32, kind="ExternalInput").ap()
    y = nc.dram_tensor("y", [PAIRS_PER_CORE, S, S], I8, kind="ExternalOutput").ap()

    n_ss = PAIRS_PER_CORE // 2  # supersteps, 2 pairs each (A on partitions 0:64, B on 64:128)
    n_mchunk = S // 128  # 16 m-chunks of 128 rows
    if repeat > 1:
        # distinct input shape per repeat-count so jax's compilation cache
        # cannot collide programs that differ only in the BIR payload
        nc.dram_tensor("rep_marker", [1, repeat], F32, kind="ExternalInput")

    with tile.TileContext(nc) as tc:
      for _rep in range(repeat):
        with (
            tc.tile_pool(name="const", bufs=1) as const_pool,
            tc.tile_pool(name="x1raw", bufs=3) as x1raw_pool,
            tc.tile_pool(name="x1i8", bufs=2) as x1i8_pool,
            tc.tile_pool(name="x1bf", bufs=2) as x1bf_pool,
            tc.tile_pool(name="x2raw", bufs=3) as x2raw_pool,
            tc.tile_pool(name="x2i8", bufs=2) as x2i8_pool,
            tc.tile_pool(name="x2bf", bufs=2) as x2bf_pool,
            tc.tile_pool(name="x1T", bufs=2) as x1T_pool,
            tc.tile_pool(name="ostage", bufs=6) as ostage_pool,
            tc.tile_pool(name="tpsum", bufs=2, space="PSUM") as tpsum_pool,
            tc.tile_pool(name="mpsum", bufs=3, space="PSUM") as mpsum_pool,
        ):
            identity = const_pool.tile([128, 128], BF16)
            make_identity(nc, identity)
            ev = {"act": 0.0, "dve": 0.0}

            def input_loads(ss):
                pa, pb = 2 * ss, 2 * ss + 1
                x2r = x2raw_pool.tile([128, S], F32, tag="x2raw")
                nc.sync.dma_start(out=x2r[0:64, :], in_=x2[pa])
                nc.sync.dma_start(out=x2r[64:128, :], in_=x2[pb])
                x1rs = []
                for p in (pa, pb):
                    x1r = x1raw_pool.tile([128, n_mchunk * D], F32, tag="x1raw")
                    nc.sync.dma_start(
                        out=x1r.rearrange("p (c d) -> p c d", c=n_mchunk),
                        in_=x1[p].rearrange("(c p) d -> p c d", p=128),
                    )
                    x1rs.append(x1r)
                return x2r, x1rs

            def assign(cost_act, cost_dve):
                # deficit-weighted ACT/DVE balancing (returns engine + books cost)
                if ev["act"] + cost_act <= ev["dve"] + cost_dve:
                    ev["act"] += cost_act
                    return "act"
                ev["dve"] += cost_dve
                return "dve"

            def prep_compute(ss, x2r, x1rs, use_pool):
                # rescale f32 -> *r -> int8 (RNE+sat), convert int8 -> bf16
                # x2 path always on GPSIMD (off the evict engines' critical path)
                x2i = x2i8_pool.tile([128, S], I8, tag="x2i8")
                x2b = x2bf_pool.tile([128, S], BF16, tag="x2bf")
                nc.gpsimd.tensor_scalar_mul(x2i[:], x2r[:], r2)
                nc.gpsimd.tensor_copy(x2b[:], x2i[:])
                x1bfs = []
                for x1r in x1rs:
                    x1i = x1i8_pool.tile([128, n_mchunk * D], I8, tag="x1i8")
                    x1b = x1bf_pool.tile([128, n_mchunk * D], BF16, tag="x1bf")
                    if use_pool:
                        nc.gpsimd.tensor_scalar_mul(x1i[:], x1r[:], r1)
                        nc.gpsimd.tensor_copy(x1b[:], x1i[:])
                    else:
                        # ss0: x1 feeds the first transposes - keep it fast
                        nc.vector.tensor_scalar_mul(x1i[:], x1r[:], r1)
                        nc.scalar.activation(x1b[:], x1i[:], AF.Copy)
                        ev["dve"] += 664.0
                        ev["act"] += 1095.0
                    x1bfs.append(x1b)
                # PE transpose x1 [128(s),64(d)] chunks -> x1T [64(d),128(s)];
                # pair A -> psum partitions 0:64 (cols 0:64), pair B -> 64:128
                x1T = x1T_pool.tile([128, S], BF16, tag="x1T")
                for g in range(n_mchunk // 4):
                    tp = tpsum_pool.tile([128, 512], BF16, tag="tpsum")
                    for j in range(4):
                        c = g * 4 + j
                        nc.tensor.transpose(
                            tp[0:64, j * 128:(j + 1) * 128],
                            x1bfs[0][:, c * D:(c + 1) * D],
                            identity[:],
                            tile_position=(0, 0),
                        )
                        nc.tensor.transpose(
                            tp[64:128, j * 128:(j + 1) * 128],
                            x1bfs[1][:, c * D:(c + 1) * D],
                            identity[:],
                            tile_position=(0, 64),
                        )
                    if assign(669.0, 462.0) == "act":
                        nc.scalar.activation(x1T[:, g * 512:(g + 1) * 512], tp[:], AF.Copy)
                    else:
                        nc.vector.tensor_copy(x1T[:, g * 512:(g + 1) * 512], tp[:])
                return x1T, x2b

            def main(ss, x1T, x2b):
                for m in range(n_mchunk):
                    for half, p in ((0, 2 * ss), (1, 2 * ss + 1)):
                        lo, hi = half * 64, half * 64 + 64
                        ost = ostage_pool.tile([128, S], I8, tag="ostage")
                        for nn in range(S // 1024):
                            ps = mpsum_pool.tile([128, 1024], F32, tag="mpsum")
                            for k in range(2):
                                n0 = nn * 1024 + k * N_MM
                                nc.tensor.matmul(
                                    ps[:, k * N_MM:(k + 1) * N_MM],
                                    lhsT=x1T[lo:hi, m * 128:(m + 1) * 128],
                                    rhs=x2b[lo:hi, n0:n0 + N_MM],
                                    start=True,
                                    stop=True,
                                    tile_position=(half * 64, 0),
                                )
                            dst = ost[:, nn * 1024:(nn + 1) * 1024]
                            # evict: *1/16 then f32->int8 (RNE+sat); deficit-
                            # weighted ACT/DVE split (ACT is cheaper per elem)
                            if assign(1095.0, 1262.0) == "act":
                                nc.scalar.activation(dst, ps[:], AF.Copy, scale=INV_G)
                            else:
                                nc.vector.tensor_scalar_mul(dst, ps[:], INV_G)
                        nc.sync.dma_start(
                            out=y[p, m * 128:(m + 1) * 128, :], in_=ost[:]
                        )

            loads0 = input_loads(0)
            p0 = prep_compute(0, *loads0, use_pool=False)
            loads1 = input_loads(1)
            main(0, *p0)
            p1 = prep_compute(1, *loads1, use_pool=True)
            main(1, *p1)

    nc.compile()
    return nc


_CACHE: dict = {}


def kernel(x1, x2, scale1_last_layer, scale_x1, scale2_last_layer, scale_x2):
    x1 = np.asarray(x1, dtype=np.float32)
    x2 = np.asarray(x2, dtype=np.float32)
    # same fp32 division the reference performs
    r1 = float(np.float32(scale1_last_layer) / np.float32(scale_x1))
    r2 = float(np.float32(scale2_last_layer) / np.float32(scale_x2))

    key = (r1, r2)
    if key not in _CACHE:
        _CACHE[key] = build_program(r1, r2)
    nc = _CACHE[key]

    b, h = x1.shape[0], x1.shape[1]
    x1r = x1.reshape(b * h, S, D)
    x2r = x2.reshape(b * h, D, S)
    in_maps = [
        {
            "x1": np.ascontiguousarray(x1r[c * PAIRS_PER_CORE:(c + 1) * PAIRS_PER_CORE]),
            "x2": np.ascontiguousarray(x2r[c * PAIRS_PER_CORE:(c + 1) * PAIRS_PER_CORE]),
        }
        for c in range(N_CORES)
    ]
    res = run_bass_kernel_spmd(nc, in_maps, list(range(N_CORES)))
    out = np.concatenate([r["y"] for r in res.results], axis=0)
    return out.reshape(b, h, S, S).astype(np.float32)


if __name__ == "__main__":
    # smoke test with random data
    rng = np.random.default_rng(0)
    x1 = np.round(np.clip(rng.normal(size=(2, 16, S, D)) * 40.0, -128, 127)).astype(np.float32)
    x2 = np.round(np.clip(rng.normal(size=(2, 16, D, S)) * 40.0, -128, 127)).astype(np.float32)
    y = kernel(x1, x2, np.float32(0.1), np.float32(0.05), np.float32(0.08), np.float32(0.04))
    print("out", y.shape, y.dtype, y[0, 0, :2, :8])


# revision 13
# speedup vs baseline: 1.4787x; 1.4787x over previous
"""Int-infer matmul kernel for trn2, 8 NeuronCores, data-parallel over (b,h).

reference: y = clip(round(matmul(clip(round(x1*r1)), clip(round(x2*r2))) / 16), -128, 127)
shapes: x1 [2,16,2048,64] f32, x2 [2,16,64,2048] f32 -> y [2,16,2048,2048] f32

Strategy (per core, 4 of the 32 (b,h) pairs):
 - rescale: f32 -> *r -> int8 (HW convert = RNE + saturate == clip(round(.)))
 - int8 -> bf16 (exact for [-128,127]); bf16 matmul accumulates exactly in f32 PSUM
 - x1 transposed on PE (col-tiled so pair A -> psum partitions 0:64, B -> 64:128)
 - main matmuls row-packed: two K=64 matmuls (pairs A,B) run concurrently via
   tile_position (0,0)/(64,0)
 - evict psum f32 -> *1/16 -> int8 (RNE+sat == clip(round(y/16))), alternating
   DVE/ACT; int8 output DMA'd out (4x fewer bytes), upcast to f32 on host
"""
import sys

sys.path.insert(0, "/opt/trn_rl_repo")

import numpy as np
import concourse.bass as bass
import concourse.bacc as bacc
import concourse.mybir as mybir
import concourse.tile as tile
from concourse.bass_utils import run_bass_kernel_spmd
from concourse.masks import make_identity

F32 = mybir.dt.float32
BF16 = mybir.dt.bfloat16
I8 = mybir.dt.int8
AF = mybir.ActivationFunctionType

N_CORES = 8
PAIRS_PER_CORE = 4  # 2*16 = 32 (b,h) pairs / 8 cores
S = 2048
D = 64
N_MM = 512  # moving free dim per matmul
INV_G = 1.0 / 16.0


def build_program(r1: float, r2: float, repeat: int = 1) -> bass.Bass:
    nc = bacc.Bacc("TRN2", target_bir_lowering=False, debug=False, num_devices=N_CORES)
    x1 = nc.dram_tensor("x1", [PAIRS_PER_CORE, S, D], F32, kind="ExternalInput").ap()
    x2 = nc.dram_tensor("x2", [PAIRS_PER_CORE, D, S], F32, kind="ExternalInput").ap()
    y = nc.dram_tensor("y", [PAIRS_PER_CORE, S, S], I8, kind="ExternalOutput").ap()

    n_ss = PAIRS_PER_CORE // 2  # supersteps, 2 pairs each (A on partitions 0:64, B on 64:128)
    n_mchunk = S // 128  # 16 m-chunks of 128 rows
    if repeat > 1:
        # distinct input shape per repeat-count so jax's compilation cache
        # cannot collide programs that differ only in the BIR payload
        nc.dram_tensor("rep_marker", [1, repeat], F32, kind="ExternalInput")

    with tile.TileContext(nc) as tc:
      for _rep in range(repeat):
        with (
            tc.tile_pool(name="const", bufs=1) as const_pool,
            tc.tile_pool(name="x1raw", bufs=3) as x1raw_pool,
            tc.tile_pool(name="x1i8", bufs=2) as x1i8_pool,
            tc.tile_pool(name="x1bf", bufs=2) as x1bf_pool,
            tc.tile_pool(name="x2raw", bufs=3) as x2raw_pool,
            tc.tile_pool(name="x2i8", bufs=2) as x2i8_pool,
            tc.tile_pool(name="x2bf", bufs=2) as x2bf_pool,
            tc.tile_pool(name="x1T", bufs=2) as x1T_pool,
            tc.tile_pool(name="ostage", bufs=6) as ostage_pool,
            tc.tile_pool(name="tpsum", bufs=2, space="PSUM") as tpsum_pool,
            tc.tile_pool(name="mpsum", bufs=3, space="PSUM") as mpsum_pool,
        ):
            identity = const_pool.tile([128, 128], BF16)
            make_identity(nc, identity)
            ev = {"act": 0.0, "dve": 0.0}

            def input_loads(ss):
                pa, pb = 2 * ss, 2 * ss + 1
                x2r = x2raw_pool.tile([128, S], F32, tag="x2raw")
                nc.sync.dma_start(out=x2r[0:64, :], in_=x2[pa])
                nc.sync.dma_start(out=x2r[64:128, :], in_=x2[pb])
                x1rs = []
                h = n_mchunk // 2
                for p in (pa, pb):
                    x1r = x1raw_pool.tile([128, n_mchunk * D], F32, tag="x1raw")
                    dst = x1r.rearrange("p (c d) -> p c d", c=n_mchunk)
                    srcv = x1[p].rearrange("(c p) d -> p c d", p=128)
                    nc.sync.dma_start(out=dst[:, 0:h, :], in_=srcv[:, 0:h, :])
                    nc.sync.dma_start(out=dst[:, h:, :], in_=srcv[:, h:, :])
                    x1rs.append(x1r)
                return x2r, x1rs

            def assign(cost_act, cost_dve):
                # deficit-weighted ACT/DVE balancing (returns engine + books cost)
                if ev["act"] + cost_act <= ev["dve"] + cost_dve:
                    ev["act"] += cost_act
                    return "act"
                ev["dve"] += cost_dve
                return "dve"

            def prep_compute(ss, x2r, x1rs, use_pool):
                # rescale f32 -> *r -> int8 (RNE+sat), convert int8 -> bf16
                # x2 path always on GPSIMD (off the evict engines' critical path)
                x2i = x2i8_pool.tile([128, S], I8, tag="x2i8")
                x2b = x2bf_pool.tile([128, S], BF16, tag="x2bf")
                nc.gpsimd.tensor_scalar_mul(x2i[:], x2r[:], r2)
                nc.gpsimd.tensor_copy(x2b[:], x2i[:])
                x1bfs = []
                for x1r in x1rs:
                    x1i = x1i8_pool.tile([128, n_mchunk * D], I8, tag="x1i8")
                    x1b = x1bf_pool.tile([128, n_mchunk * D], BF16, tag="x1bf")
                    if use_pool:
                        nc.gpsimd.tensor_scalar_mul(x1i[:], x1r[:], r1)
                        nc.gpsimd.tensor_copy(x1b[:], x1i[:])
                    else:
                        # ss0: x1 feeds the first transposes - keep it fast
                        nc.vector.tensor_scalar_mul(x1i[:], x1r[:], r1)
                        nc.scalar.activation(x1b[:], x1i[:], AF.Copy)
                        ev["dve"] += 664.0
                        ev["act"] += 1095.0
                    x1bfs.append(x1b)
                # PE transpose x1 [128(s),64(d)] chunks -> x1T [64(d),128(s)];
                # pair A -> psum partitions 0:64 (cols 0:64), pair B -> 64:128.
                # One x1T tile per 4-chunk group so main matmuls for m-chunks
                # 4g..4g+3 depend only on group g's copy (earlier MM start).
                x1Ts = []
                for g in range(n_mchunk // 4):
                    tp = tpsum_pool.tile([128, 512], BF16, tag="tpsum")
                    for j in range(4):
                        c = g * 4 + j
                        nc.tensor.transpose(
                            tp[0:64, j * 128:(j + 1) * 128],
                            x1bfs[0][:, c * D:(c + 1) * D],
                            identity[:],
                            tile_position=(0, 0),
                        )
                        nc.tensor.transpose(
                            tp[64:128, j * 128:(j + 1) * 128],
                            x1bfs[1][:, c * D:(c + 1) * D],
                            identity[:],
                            tile_position=(0, 64),
                        )
                    x1T = x1T_pool.tile([128, 512], BF16, tag=f"x1T{g}")
                    if assign(669.0, 462.0) == "act":
                        nc.scalar.activation(x1T[:], tp[:], AF.Copy)
                    else:
                        nc.vector.tensor_copy(x1T[:], tp[:])
                    x1Ts.append(x1T)
                return x1Ts, x2b

            def main(ss, x1Ts, x2b):
                for mp in range(n_mchunk // 2):
                    for half, p in ((0, 2 * ss), (1, 2 * ss + 1)):
                        lo, hi = half * 64, half * 64 + 64
                        # one staging tile + one output DMA covers 2 m-chunks
                        ost = ostage_pool.tile([128, 2 * S], I8, tag="ostage")
                        for mm in range(2):
                            m = 2 * mp + mm
                            x1T = x1Ts[m // 4]
                            moff = (m % 4) * 128
                            for nn in range(S // 1024):
                                ps = mpsum_pool.tile([128, 1024], F32, tag="mpsum")
                                for k in range(2):
                                    n0 = nn * 1024 + k * N_MM
                                    nc.tensor.matmul(
                                        ps[:, k * N_MM:(k + 1) * N_MM],
                                        lhsT=x1T[lo:hi, moff:moff + 128],
                                        rhs=x2b[lo:hi, n0:n0 + N_MM],
                                        start=True,
                                        stop=True,
                                        tile_position=(half * 64, 0),
                                    )
                                dst = ost[:, mm * S + nn * 1024:mm * S + (nn + 1) * 1024]
                                # evict: *1/16 then f32->int8 (RNE+sat); deficit-
                                # weighted ACT/DVE split (ACT cheaper per elem)
                                if assign(1095.0, 1262.0) == "act":
                                    nc.scalar.activation(dst, ps[:], AF.Copy, scale=INV_G)
                                else:
                                    nc.vector.tensor_scalar_mul(dst, ps[:], INV_G)
                        nc.sync.dma_start(
                            out=y[p, 2 * mp * 128:(2 * mp + 2) * 128, :].rearrange(
                                "(r p) c -> p r c", p=128
                            ),
                            in_=ost.rearrange("p (r c) -> p r c", r=2),
                        )

            loads0 = input_loads(0)
            p0 = prep_compute(0, *loads0, use_pool=False)
            loads1 = input_loads(1)
            main(0, *p0)
            p1 = prep_compute(1, *loads1, use_pool=True)
            main(1, *p1)

    nc.compile()
    return nc


_CACHE: dict = {}


def kernel(x1, x2, scale1_last_layer, scale_x1, scale2_last_layer, scale_x2):
    x1 = np.asarray(x1, dtype=np.float32)
    x2 = np.asarray(x2, dtype=np.float32)
    # same fp32 division the reference performs
    r1 = float(np.float32(scale1_last_layer) / np.float32(scale_x1))
    r2 = float(np.float32(scale2_last_layer) / np.float32(scale_x2))

    key = (r1, r2)
    if key not in _CACHE:
        _CACHE[key] = build_program(r1, r2)
    nc = _CACHE[key]

    b, h = x1.shape[0], x1.shape[1]
    x1r = x1.reshape(b * h, S, D)
    x2r = x2.reshape(b * h, D, S)
    in_maps = [
        {
            "x1": np.ascontiguousarray(x1r[c * PAIRS_PER_CORE:(c + 1) * PAIRS_PER_CORE]),
            "x2": np.ascontiguousarray(x2r[c * PAIRS_PER_CORE:(c + 1) * PAIRS_PER_CORE]),
        }
        for c in range(N_CORES)
    ]
    res = run_bass_kernel_spmd(nc, in_maps, list(range(N_CORES)))
    out = np.concatenate([r["y"] for r in res.results], axis=0)
    return out.reshape(b, h, S, S).astype(np.float32)


if __name__ == "__main__":
    # smoke test with random data
    rng = np.random.default_rng(0)
    x1 = np.round(np.clip(rng.normal(size=(2, 16, S, D)) * 40.0, -128, 127)).astype(np.float32)
    x2 = np.round(np.clip(rng.normal(size=(2, 16, D, S)) * 40.0, -128, 127)).astype(np.float32)
    y = kernel(x1, x2, np.float32(0.1), np.float32(0.05), np.float32(0.08), np.float32(0.04))
    print("out", y.shape, y.dtype, y[0, 0, :2, :8])
